# revision 15
# baseline (speedup 1.0000x reference)
"""KSparseFFTClassifier Trainium2 kernel.

Math: reference computes
    h   = x @ W_proj.T + b_proj                      (bs, 129)
    h  *= scale  (sqrt(2) on dims 1..64)
    out = IDFT65(h[:, :65]) + h[:, 65:] @ Ws.T       (bs, 16384)

The zero-padded orthonormal IDFT of the 65 nonzero frequency components is a
dense matmul against a (65, N) cos/sin basis; the DC row of that basis is the
constant 1/sqrt(N).  So with M = [scaled cos/sin basis for h dims 1..64;
Ws.T]  (128 x N):

    out[b, n] = h[b, 1:129] @ M[:, n] + (h[b, 0] + b0) / sqrt(N)

The device computes the rank-128 part (all the O(BS*N*D) work); the rank-1
DC column (x @ W_proj[0] + b0)/sqrt(N) — 0.006% of the FLOPs — is applied
on the host during unpacking.  Centering the output this way shrinks its
dynamic range from mean -6.24, std 1 to mean 0, std 1, so the device can
store it in fp8 e4m3 (quantization rel err 4.2e-3 vs the 2e-2 budget) and
output traffic halves: per-core DMA is 6.5 MB in (fp16) + 8 MB out (fp8).

Sharding: data-parallel over batch, 512 rows per core on 8 cores.

Schedule:
 - input loads (wt, consts, x j-blocks, mmat) all on the ACT HWDGE queue in
   priority order; stores on the SP queue.  (Two concurrent queues share
   DMA engines by row size and would starve matmul1 of x.)
 - 4 garbage warmup matmuls ramp the PE clock while wt/x are in flight.
 - matmul1 runs per 128-row batch block (j-major) so hT[:, j=0] is ready
   ~5us after x block 0 lands and matmul2 starts ~6us earlier than a
   batch-wide matmul1 would allow.  matmul1 for block j+1 is emitted into
   the PE stream shortly after block j's matmul2 begins.
 - matmul2 accumulates 2x [128,512] into [128,1024] (2-bank) PSUM tiles
   (pool of 3); eviction casts f32->fp8 in [128,1024] chunks, greedily
   balanced between ACT (~1.11us) and DVE (~1.22us), into 8 rotating
   [128,4096] fp8 output tiles stored on SP.  The final tile is split so
   its first half's store overlaps the second half's eviction.
"""

import numpy as np

BS = 4096
IN_DIM = 2048
N = 16384
K = 32
SLACK = 64
NCORES = 8
BC = BS // NCORES        # 512 batch rows per core
P = 128
KT = IN_DIM // P         # 16 contraction tiles for matmul1
NJ = BC // P             # 4 batch blocks per core
NCHUNK = 4096            # output column chunk (SBUF out tile free size)
NCH = N // NCHUNK        # 4
EV = 1024                # eviction chunk (2 PSUM banks)
WARMUP = 4
ACT_NS = 1114            # measured eviction cost per EV chunk
DVE_NS = 1224

MM1_DT = "float16"
MM2_DT = "float16"
OUT_DT = "float8e4"      # centered output; host adds the DC column back

_NC_CACHE = {}


def _build_nc(mm1_name, mm2_name):
    import concourse.bacc as bacc
    import concourse.mybir as mybir
    import concourse.tile as tile

    class _SlimTileContext(tile.TileContext):
        """Minimal epilogue: keep only the SP drain with its DMA-completion
        waits (output correctness); skip the all-engine barriers and the
        per-sem clear (NEFF is loaded fresh per execution here)."""

        def _drain_and_barrier(self, tick_clock, wait_clock):
            from concourse.vector_clock import ScopedClock
            drain_inst = self.nc.sync.drain()
            wait_clock.add_sem_waits(
                drain_inst.ins, ScopedClock({None: tick_clock.global_clock})
            )
            popped = self.nc._tile_sem_poison_stack.pop()
            assert popped is self._sem_poison

    f32 = mybir.dt.float32
    mm1 = getattr(mybir.dt, mm1_name)
    mm2 = getattr(mybir.dt, mm2_name)
    odt = getattr(mybir.dt, OUT_DT)

    nc = bacc.Bacc("TRN2", target_bir_lowering=False)

    wt = nc.dram_tensor("wt", [P, KT * P], mm1, kind="ExternalInput")
    # xT packed j-major: [P, j*KT*P + kt*P + b%P]
    xT = nc.dram_tensor("xT", [P, KT * BC], mm1, kind="ExternalInput")
    mmat = nc.dram_tensor("mmat", [P, N], mm2, kind="ExternalInput")
    consts = nc.dram_tensor("consts", [P, 1], f32, kind="ExternalInput")
    out = nc.dram_tensor("out", [BC, N], odt, kind="ExternalOutput")

    with _SlimTileContext(nc) as tc:
        with (
            tc.tile_pool(name="wp", bufs=1) as wp,
            tc.tile_pool(name="xp", bufs=1) as xp,
            tc.tile_pool(name="mp", bufs=1) as mp,
            tc.tile_pool(name="hp", bufs=1) as hp,
            tc.tile_pool(name="op", bufs=8) as op,
            tc.tile_pool(name="ps", bufs=3, space="PSUM") as ps,
            tc.tile_pool(name="ps1", bufs=1, space="PSUM") as ps1,
        ):
            # input loads, in priority order, all on the ACT HWDGE queue
            wt_sb = wp.tile([P, KT * P], mm1, tag="wt")
            nc.scalar.dma_start(out=wt_sb[:, :], in_=wt[:, :])
            cst_sb = wp.tile([P, 1], f32, tag="cst")
            nc.scalar.dma_start(out=cst_sb[:, :], in_=consts[:, :])
            xg = []
            for g in range(NJ):
                t = xp.tile([P, KT * P], mm1, tag=f"xg{g}")
                nc.scalar.dma_start(
                    out=t[:, :], in_=xT[:, g * KT * P:(g + 1) * KT * P])
                xg.append(t)
            mm = []
            for ti in range(NCH):
                m = mp.tile([P, NCHUNK], mm2, tag=f"m{ti}")
                nc.scalar.dma_start(
                    out=m[:, :], in_=mmat[:, ti * NCHUNK:(ti + 1) * NCHUNK])
                mm.append(m)

            # PE clock warmup on garbage-initialized scratch
            scr_sb = wp.tile([P, 512], mm1, tag="scr")
            nc.gpsimd.memset(scr_sb[:, :], 0.0)
            hT_ps = ps1.tile([P, BC], f32, tag="hT")
            for w in range(WARMUP):
                nc.tensor.matmul(
                    hT_ps[:, :],
                    lhsT=scr_sb[:, 0:P],
                    rhs=scr_sb[:, 0:512],
                    start=True,
                    stop=True,
                )

            hT_sb = hp.tile([P, BC], mm2, tag="hT_sb")

            def mm1_block(j):
                for kt in range(KT):
                    nc.tensor.matmul(
                        hT_ps[:, j * P:(j + 1) * P],
                        lhsT=wt_sb[:, kt * P:(kt + 1) * P],
                        rhs=xg[j][:, kt * P:(kt + 1) * P],
                        start=(kt == 0),
                        stop=(kt == KT - 1),
                    )
                nc.scalar.add(hT_sb[:, j * P:(j + 1) * P],
                              hT_ps[:, j * P:(j + 1) * P], cst_sb[:, 0:1])

            mm1_block(0)

            # matmul2 + fp8 eviction (greedy ACT/DVE balance) + store
            act_t = dve_t = 0
            for j in range(NJ):
                for ti in range(NCH):
                    # emit next batch block's matmul1 into the PE stream just
                    # after this block's matmul2 begins
                    if ti == 1 and j + 1 < NJ:
                        mm1_block(j + 1)
                    last = (ti == NCH - 1) and (j == NJ - 1)
                    parts = 2 if last else 1
                    pw = NCHUNK // parts
                    for pi in range(parts):
                        ob = op.tile([P, pw], odt, tag="ob")
                        for c in range(pw // EV):
                            pt = ps.tile([P, EV], f32, tag="mm2")
                            for s in range(EV // 512):
                                col = pi * pw + c * EV + s * 512
                                nc.tensor.matmul(
                                    pt[:, s * 512:(s + 1) * 512],
                                    lhsT=hT_sb[:, j * P:(j + 1) * P],
                                    rhs=mm[ti][:, col:col + 512],
                                    start=True,
                                    stop=True,
                                )
                            dst = ob[:, c * EV:(c + 1) * EV]
                            if act_t + ACT_NS <= dve_t + DVE_NS:
                                nc.scalar.copy(dst, pt[:, :])
                                act_t += ACT_NS
                            else:
                                nc.vector.tensor_scalar_add(dst, pt[:, :], 0.0)
                                dve_t += DVE_NS
                        nc.sync.dma_start(
                            out=out[j * P:(j + 1) * P,
                                    ti * NCHUNK + pi * pw:ti * NCHUNK + (pi + 1) * pw],
                            in_=ob[:, :],
                        )
    nc.compile()
    return nc


def _get_nc():
    key = (MM1_DT, MM2_DT)
    if key not in _NC_CACHE:
        _NC_CACHE[key] = _build_nc(*key)
    return _NC_CACHE[key]


def _np_dt(name):
    import ml_dtypes
    return {"float16": np.float16, "bfloat16": ml_dtypes.bfloat16,
            "float32": np.float32, "float32r": np.float32}[name]


def _host_pack(x, W_proj, b_proj, Ws):
    dt1 = _np_dt(MM1_DT)
    dt2 = _np_dt(MM2_DT)
    SQRT2 = np.float64(np.sqrt(np.float32(2.0)))
    n_idx = np.arange(N, dtype=np.float64)
    k_idx = np.arange(1, K + 1, dtype=np.float64)
    theta = (2.0 * np.pi / N) * np.outer(k_idx, n_idx)
    M = np.empty((P, N), np.float32)
    isqn = 1.0 / np.sqrt(np.float64(N))
    M[0:2 * K:2] = (SQRT2 * isqn) * np.cos(theta)
    M[1:2 * K:2] = (SQRT2 * isqn) * np.sin(theta)
    M[2 * K:] = Ws.T
    M = M.astype(dt2)

    w1 = W_proj[1:P + 1]                                  # (128, 2048)
    wt = np.ascontiguousarray(
        w1.T.reshape(KT, P, P).transpose(1, 0, 2).reshape(P, KT * P)
    ).astype(dt1)

    # DC column on host: (x @ W_proj[0] + b0) / sqrt(N)
    dc_all = ((x.astype(np.float64) @ W_proj[0].astype(np.float64)
               + np.float64(b_proj[0])) * isqn).astype(np.float32)

    cst = np.ascontiguousarray(
        b_proj[1:P + 1].astype(np.float32).reshape(P, 1))
    xts = []
    for c in range(NCORES):
        xc = x[c * BC:(c + 1) * BC]                        # (512, 2048)
        # j-major: [P, j, KT, P] with partition = input-dim block row
        xt = np.ascontiguousarray(
            xc.T.reshape(KT, P, NJ, P).transpose(1, 2, 0, 3).reshape(P, KT * BC)
        ).astype(dt1)
        xts.append(xt)
    return M, wt, cst, xts, dc_all


def kernel(x, W_proj, b_proj, Ws, _trace=False, _tmpdir=None):
    from concourse import bass_utils

    x = np.ascontiguousarray(x, np.float32)
    W_proj = np.ascontiguousarray(W_proj, np.float32)
    b_proj = np.ascontiguousarray(b_proj, np.float32)
    Ws = np.ascontiguousarray(Ws, np.float32)

    M, wt, cst, xts, dc_all = _host_pack(x, W_proj, b_proj, Ws)
    nc = _get_nc()

    in_maps = [
        {"xT": xts[c], "wt": wt, "mmat": M, "consts": cst}
        for c in range(NCORES)
    ]
    kw = {}
    if _trace:
        kw = dict(trace=True, tmpdir=_tmpdir, trace_cores=[0])
    res = bass_utils.run_bass_kernel_spmd(nc, in_maps, core_ids=list(range(NCORES)), **kw)
    import ml_dtypes
    lut = np.arange(256, dtype=np.uint8).view(ml_dtypes.float8_e4m3fn).astype(np.float32)
    outs = []
    for c, r in enumerate(res.results):
        o = lut[r["out"].view(np.uint8)]
        o += dc_all[c * BC:(c + 1) * BC, None]
        outs.append(o)
    out = np.concatenate(outs, axis=0)
    if _trace:
        return out, res
    return out
